# revision 11
# baseline (speedup 1.0000x reference)
"""Trainium2 Bass kernel for BiomarkerCombinationFinder.

Strategy: shard the combination axis M (padded 100->104) across 8 cores,
13 combos per core; replicate the batch (128) on SBUF partitions.  Per
combo: logits matmul on PE (fp32), gumbel-softmax stats on ACT/DVE,
top-5 via the DVE Max/MaxIndex custom ops, scorer/synergy MLPs fed by
dma_gather of the needed weight rows (2KB each) and x values.
"""

import os
import sys

sys.path.insert(0, "/opt/trn_rl_repo")

import numpy as np
from contextlib import ExitStack

import concourse.bass as bass
import concourse.bacc as bacc
import concourse.mybir as mybir
import concourse.tile as tile
from concourse.bass_utils import run_bass_kernel_spmd

B, D, H, M, K = 128, 1024, 256, 100, 5
NCORES = 8
MP = 104          # M padded to a multiple of NCORES
MC = MP // NCORES # combos per core
TAU = 0.5
EPS = 1e-5

F32 = mybir.dt.float32
I16 = mybir.dt.int16
U16 = mybir.dt.uint16

AF = mybir.ActivationFunctionType
AL = mybir.AluOpType
AX = mybir.AxisListType

_CACHE: dict = {}


def _build_nc():
    n_m = int(os.environ.get("K_NM", str(MC)))
    no_scorer = bool(int(os.environ.get("K_NOSCORER", "0")))
    no_topk = bool(int(os.environ.get("K_NOTOPK", "0")))
    no_h = bool(int(os.environ.get("K_NOH", "0")))
    no_mm = bool(int(os.environ.get("K_NOMM", "0")))
    nc = bacc.Bacc(trn_type="TRN2", num_devices=NCORES)

    # ---- DRAM I/O ----
    xT = nc.dram_tensor("xT", [D, B], F32, kind="ExternalInput")
    g2 = nc.dram_tensor("g2", [B, MC, D], F32, kind="ExternalInput")       # 2*(gumbel+gb3) slice
    w3 = nc.dram_tensor("w3", [MC, 2, 128, D], F32, kind="ExternalInput")  # gW3 slice, tiled
    wsy = nc.dram_tensor("wsy", [K * D, 2 * H], F32, kind="ExternalInput") # [sW1|yW1]
    gw1 = nc.dram_tensor("gw1", [D, 2 * H], F32, kind="ExternalInput")
    gw2 = nc.dram_tensor("gw2", [2 * H, H], F32, kind="ExternalInput")
    sw2 = nc.dram_tensor("sw2", [H, H // 2], F32, kind="ExternalInput")
    # replicated-across-partitions vectors
    gb1r = nc.dram_tensor("gb1r", [B, 2 * H], F32, kind="ExternalInput")
    ln1sr = nc.dram_tensor("ln1sr", [B, 2 * H], F32, kind="ExternalInput")
    ln1br = nc.dram_tensor("ln1br", [B, 2 * H], F32, kind="ExternalInput")
    gb2r = nc.dram_tensor("gb2r", [B, H], F32, kind="ExternalInput")
    ln2sr = nc.dram_tensor("ln2sr", [B, H], F32, kind="ExternalInput")
    ln2br = nc.dram_tensor("ln2br", [B, H], F32, kind="ExternalInput")
    sb1yr = nc.dram_tensor("sb1yr", [B, 2 * H], F32, kind="ExternalInput")
    sb2r = nc.dram_tensor("sb2r", [B, H // 2], F32, kind="ExternalInput")
    sw3r = nc.dram_tensor("sw3r", [B, H // 2], F32, kind="ExternalInput")
    yw2r = nc.dram_tensor("yw2r", [B, H], F32, kind="ExternalInput")
    sb3r = nc.dram_tensor("sb3r", [B, 1], F32, kind="ExternalInput")
    yb2r = nc.dram_tensor("yb2r", [B, 1], F32, kind="ExternalInput")

    out_l = nc.dram_tensor("out_l", [B, MC, D], F32, kind="ExternalOutput")
    out_p = nc.dram_tensor("out_p", [B, MC, D], F32, kind="ExternalOutput")
    out_s = nc.dram_tensor("out_s", [B, MC], F32, kind="ExternalOutput")
    out_y = nc.dram_tensor("out_y", [B, MC], F32, kind="ExternalOutput")

    with tile.TileContext(nc) as tc, ExitStack() as ctx:
        consts = ctx.enter_context(tc.tile_pool(name="consts", bufs=1))

        # ---- constants in SBUF ----
        ones = consts.tile([128, 640], F32, tag="ones")
        nc.vector.memset(ones[:], 1.0)

        ident = consts.tile([128, 128], F32, tag="ident")
        nc.gpsimd.affine_select(
            ident[:], ones[:, 0:128], pattern=[[1, 128]], base=0,
            channel_multiplier=-1, compare_op=AL.is_equal, fill=0.0,
        )
        # I5[p, j, d] = 1 when d == p  (for extracting vals diagonal)
        i5 = consts.tile([128, 5, 128], F32, tag="i5")
        nc.gpsimd.affine_select(
            i5[:], ones[:].rearrange("p (a b) -> p a b", a=5),
            pattern=[[0, 5], [1, 128]], base=0,
            channel_multiplier=-1, compare_op=AL.is_equal, fill=0.0,
        )
        # Prr[k, f] = 1 when k == 16*rr + f%16 (partition permutations for
        # building the 16-wrapped dma_gather index layout on-chip)
        prr = []
        for rr in range(8):
            p_t = consts.tile([128, 128], F32, tag=f"prr{rr}")
            nc.gpsimd.affine_select(
                p_t[:].rearrange("p (a b) -> p a b", b=16),
                ones[:, 0:128].rearrange("p (a b) -> p a b", b=16),
                pattern=[[0, 8], [-1, 16]], base=-16 * rr,
                channel_multiplier=1, compare_op=AL.is_equal, fill=0.0,
            )
            prr.append(p_t)
        # iota_off[p, j*8+rr] = j*1024
        iota_off = consts.tile([128, 40], I16, tag="iota_off")
        nc.gpsimd.iota(iota_off[:], pattern=[[1024, 5], [0, 8]], base=0,
                       channel_multiplier=0)

        # ---- weights/bias tiles ----
        xt_t = []
        for i in range(8):
            t_ = consts.tile([128, 128], F32, tag=f"xt{i}")
            nc.sync.dma_start(t_[:], xT.ap().rearrange("(a p) b -> a p b", p=128)[i])
            xt_t.append(t_)
        gw1_t = []
        for i in range(8):
            t_ = consts.tile([128, 2 * H], F32, tag=f"gw1_{i}")
            nc.sync.dma_start(t_[:], gw1.ap().rearrange("(a p) b -> a p b", p=128)[i])
            gw1_t.append(t_)
        gw2_t = []
        for i in range(4):
            t_ = consts.tile([128, H], F32, tag=f"gw2_{i}")
            nc.sync.dma_start(t_[:], gw2.ap().rearrange("(a p) b -> a p b", p=128)[i])
            gw2_t.append(t_)
        sw2_t = []
        for i in range(2):
            t_ = consts.tile([128, H // 2], F32, tag=f"sw2_{i}")
            nc.sync.dma_start(t_[:], sw2.ap().rearrange("(a p) b -> a p b", p=128)[i])
            sw2_t.append(t_)

        def _load(handle, shape, tag):
            t_ = consts.tile(shape, F32, tag=tag)
            nc.sync.dma_start(t_[:], handle.ap())
            return t_

        gb1r_t = _load(gb1r, [128, 2 * H], "gb1r")
        ln1sr_t = _load(ln1sr, [128, 2 * H], "ln1sr")
        ln1br_t = _load(ln1br, [128, 2 * H], "ln1br")
        gb2r_t = _load(gb2r, [128, H], "gb2r")
        ln2sr_t = _load(ln2sr, [128, H], "ln2sr")
        ln2br_t = _load(ln2br, [128, H], "ln2br")
        sb1yr_t = _load(sb1yr, [128, 2 * H], "sb1yr")
        sb2r_t = _load(sb2r, [128, H // 2], "sb2r")
        sw3r_t = _load(sw3r, [128, H // 2], "sw3r")
        yw2r_t = _load(yw2r, [128, H], "yw2r")
        sb3r_t = _load(sb3r, [128, 1], "sb3r")
        yb2r_t = _load(yb2r, [128, 1], "yb2r")

        # staging for scores / synergies (batched sigmoid/tanh at the end)
        sstage = consts.tile([128, MC], F32, tag="sstage")
        ystage = consts.tile([128, MC], F32, tag="ystage")

        epsb = consts.tile([128, 1], F32, tag="epsb")
        nc.vector.memset(epsb[:], EPS)

        # ---- generator front: h2 = relu(LN(relu(LN(x@gW1+gb1))@gW2+gb2)) ----
        def layer_norm(pool, spool, h_in, n, sr_t, br_t, tag):
            ssum = spool.tile([128, 1], F32, tag=f"{tag}_ssum")
            nc.vector.reduce_sum(ssum[:], h_in[:], axis=AX.X)
            mu = spool.tile([128, 1], F32, tag=f"{tag}_mu")
            nc.vector.tensor_scalar_mul(mu[:], ssum[:], 1.0 / n)
            hm = pool.tile([128, n], F32, tag=f"{tag}_hm")
            nc.vector.tensor_scalar(hm[:], h_in[:], mu[:], None, op0=AL.subtract)
            sq = pool.tile([128, n], F32, tag=f"{tag}_sq")
            ssq = spool.tile([128, 1], F32, tag=f"{tag}_ssq")
            nc.vector.tensor_mul(sq[:], hm[:], hm[:])
            nc.vector.reduce_sum(ssq[:], sq[:], axis=AX.X)
            std = spool.tile([128, 1], F32, tag=f"{tag}_std")
            nc.scalar.activation(std[:], ssq[:], AF.Sqrt, bias=epsb[:], scale=1.0 / n)
            rstd = spool.tile([128, 1], F32, tag=f"{tag}_rstd")
            nc.vector.reciprocal(rstd[:], std[:])
            hn = pool.tile([128, n], F32, tag=f"{tag}_hn")
            nc.vector.tensor_scalar_mul(hn[:], hm[:], rstd[:])
            hs = pool.tile([128, n], F32, tag=f"{tag}_hs")
            nc.vector.tensor_mul(hs[:], hn[:], sr_t[:])
            hb = pool.tile([128, n], F32, tag=f"{tag}_hb")
            nc.vector.tensor_add(hb[:], hs[:], br_t[:])
            hr = pool.tile([128, n], F32, tag=f"{tag}_hr")
            nc.vector.tensor_scalar_max(hr[:], hb[:], 0.0)
            return hr

        if no_h:
            h2t = []
            for i in range(2):
                st = consts.tile([128, 128], F32, tag=f"h2t{i}")
                nc.vector.memset(st[:], 0.01)
                h2t.append(st)
        else:
         with tc.tile_pool(name="hpool", bufs=1) as hpool, \
             tc.tile_pool(name="hspool", bufs=1) as hspool, \
             tc.tile_pool(name="hpsum", bufs=1, space="PSUM") as hpsum, \
             tc.tile_pool(name="htpsum", bufs=2, space="PSUM") as htpsum:
            ph1 = hpsum.tile([128, 2 * H], F32, tag="ph1")
            for i in range(8):
                nc.tensor.matmul(ph1[:], xt_t[i][:], gw1_t[i][:],
                                 start=(i == 0), stop=(i == 7))
            h1 = hpool.tile([128, 2 * H], F32, tag="h1")
            nc.vector.tensor_add(h1[:], ph1[:], gb1r_t[:])
            h1r = layer_norm(hpool, hspool, h1, 2 * H, ln1sr_t, ln1br_t, "ln1")

            h1rt = []
            for i in range(4):
                pt = htpsum.tile([128, 128], F32, tag="htp")
                nc.tensor.matmul(pt[:], h1r[:, i * 128:(i + 1) * 128], ident[:], start=True, stop=True)
                st = consts.tile([128, 128], F32, tag=f"h1rt{i}")
                nc.vector.tensor_copy(st[:], pt[:])
                h1rt.append(st)

            ph2 = hpsum.tile([128, H], F32, tag="ph2")
            for i in range(4):
                nc.tensor.matmul(ph2[:], h1rt[i][:], gw2_t[i][:],
                                 start=(i == 0), stop=(i == 3))
            h2 = hpool.tile([128, H], F32, tag="h2")
            nc.vector.tensor_add(h2[:], ph2[:], gb2r_t[:])
            h2r = layer_norm(hpool, hspool, h2, H, ln2sr_t, ln2br_t, "ln2")

            h2t = []
            for i in range(2):
                pt = htpsum.tile([128, 128], F32, tag="htp2")
                nc.tensor.matmul(pt[:], h2r[:, i * 128:(i + 1) * 128], ident[:], start=True, stop=True)
                st = consts.tile([128, 128], F32, tag=f"h2t{i}")
                nc.vector.tensor_copy(st[:], pt[:])
                h2t.append(st)

        # ---- main loop over combos ----
        wpool = ctx.enter_context(tc.tile_pool(name="wpool", bufs=2))
        gpool = ctx.enter_context(tc.tile_pool(name="gpool", bufs=2))
        lpool = ctx.enter_context(tc.tile_pool(name="lpool", bufs=2))
        tpool = ctx.enter_context(tc.tile_pool(name="tpool", bufs=2))
        epool = ctx.enter_context(tc.tile_pool(name="epool", bufs=2))
        ppool = ctx.enter_context(tc.tile_pool(name="ppool", bufs=2))
        wgpool = ctx.enter_context(tc.tile_pool(name="wgpool", bufs=2))
        xgpool = ctx.enter_context(tc.tile_pool(name="xgpool", bufs=2))
        s1pool = ctx.enter_context(tc.tile_pool(name="s1pool", bufs=2))
        spool = ctx.enter_context(tc.tile_pool(name="spool", bufs=3))
        plpsum = ctx.enter_context(tc.tile_pool(name="plpsum", bufs=2, space="PSUM"))
        smpsum = ctx.enter_context(tc.tile_pool(name="smpsum", bufs=3, space="PSUM"))

        for m in range(n_m):
            # logits = h2 @ gW3[:, m]  -> PSUM [128, 1024]
            w3t = wpool.tile([128, 2, D], F32, tag="w3t")
            for kc in range(2):
                nc.sync.dma_start(w3t[:, kc, :], w3.ap()[m, kc])
            g2t = gpool.tile([128, D], F32, tag="g2t")
            nc.sync.dma_start(g2t[:], g2.ap()[:, m, :])

            pl = plpsum.tile([128, D], F32, tag="pl")
            if no_mm:
                nc.vector.memset(pl[:], 0.25)
            else:
                for dc in range(2):
                    for kc in range(2):
                        nc.tensor.matmul(
                            pl[:, dc * 512:(dc + 1) * 512],
                            h2t[kc][:],
                            w3t[:, kc, dc * 512:(dc + 1) * 512],
                            start=(kc == 0), stop=(kc == 1),
                        )
            lsb = lpool.tile([128, D], F32, tag="lsb")
            nc.vector.tensor_copy(lsb[:], pl[:])
            nc.sync.dma_start(out_l.ap()[:, m, :], lsb[:])

            # t = 2*logits + 2*(gumbel+gb3)
            t_t = tpool.tile([128, D], F32, tag="t_t")
            nc.vector.scalar_tensor_tensor(
                t_t[:], pl[:], 2.0, g2t[:], op0=AL.mult, op1=AL.add)

            # top-8 (we use top-5)
            mx8 = spool.tile([128, 8], F32, tag="mx8")
            nc.vector.max(mx8[:], t_t[:])
            idx8 = spool.tile([128, 8], U16, tag="idx8")
            if no_topk:
                nc.vector.memset(idx8[:], 0)
            else:
                nc.vector.max_index(idx8[:], mx8[:], t_t[:])

            # softmax
            nmax = spool.tile([128, 1], F32, tag="nmax")
            nc.vector.tensor_scalar_mul(nmax[:], mx8[:, 0:1], -1.0)
            e_t = epool.tile([128, D], F32, tag="e_t")
            se = spool.tile([128, 1], F32, tag="se")
            nc.scalar.activation(e_t[:], t_t[:], AF.Exp, bias=nmax[:], accum_out=se[:])
            rc = spool.tile([128, 1], F32, tag="rc")
            nc.vector.reciprocal(rc[:], se[:])
            pb = ppool.tile([128, D], F32, tag="pb")
            nc.vector.tensor_scalar_mul(pb[:], e_t[:], rc[:])
            nc.sync.dma_start(out_p.ap()[:, m, :], pb[:])

            if no_scorer:
                nc.vector.memset(ystage[:, m:m + 1], 0.0)
                nc.vector.memset(sstage[:, m:m + 1], 0.0)
                continue

            # wrapped idx layout via one-hot permutation matmuls
            idx8f = spool.tile([128, 8], F32, tag="idx8f")
            nc.vector.tensor_copy(idx8f[:], idx8[:])
            pperm = smpsum.tile([128, 8, 8], F32, tag="smp")
            for rr in range(8):
                nc.tensor.matmul(pperm[:, rr, :], prr[rr][:], idx8f[:],
                                 start=True, stop=True)
            widx = spool.tile([128, 5, 8], I16, tag="widx")
            nc.vector.tensor_copy(
                widx[:], pperm[:].rearrange("p a b -> p b a")[:, 0:5, :])
            wpos = spool.tile([128, 40], I16, tag="wpos")
            nc.vector.tensor_add(
                wpos[:], widx[:].rearrange("p a b -> p (a b)"), iota_off[:])

            # gathers
            wg = wgpool.tile([128, 5, 2 * H], F32, tag="wg")
            nc.gpsimd.dma_gather(
                wg[:], wsy.ap(), wpos[:], num_idxs=640, num_idxs_reg=640,
                elem_size=2 * H, queue_num=0)
            xg = xgpool.tile([128, 5, 128], F32, tag="xg")
            nc.gpsimd.dma_gather(
                xg[:], xT.ap(), widx[:].rearrange("p a b -> p (a b)"),
                num_idxs=640, num_idxs_reg=640, elem_size=128, queue_num=0)

            # vals[b, j] = x[b, idx[b, j]]
            xm = xgpool.tile([128, 5, 128], F32, tag="xm")
            nc.vector.tensor_mul(xm[:], xg[:], i5[:])
            vals = spool.tile([128, 5], F32, tag="vals")
            nc.vector.reduce_sum(vals[:], xm[:], axis=AX.X)

            # s1y = relu(sum_j vals_j * W[pos_j] + [sb1|yb1])
            acc = s1pool.tile([128, 2 * H], F32, tag="acc")
            for j in range(5):
                nc.vector.scalar_tensor_tensor(
                    acc[:], wg[:, j, :], vals[:, j:j + 1],
                    (sb1yr_t[:] if j == 0 else acc[:]),
                    op0=AL.mult, op1=AL.add)
            s1y = s1pool.tile([128, 2 * H], F32, tag="s1y")
            nc.vector.tensor_scalar_max(s1y[:], acc[:], 0.0)

            # synergy head: ysum = y1 @ yW2  (per-partition dot)
            qy = s1pool.tile([128, H], F32, tag="qy")
            nc.vector.tensor_mul(qy[:], s1y[:, H:2 * H], yw2r_t[:])
            nc.vector.reduce_sum(ystage[:, m:m + 1], qy[:], axis=AX.X)

            # score head: s2 = relu(s1 @ sW2 + sb2); ssum = s2 @ sW3
            s1t = []
            for i in range(2):
                pt = smpsum.tile([128, 128], F32, tag="smp")
                nc.tensor.matmul(pt[:], s1y[:, i * 128:(i + 1) * 128], ident[:], start=True, stop=True)
                st = spool.tile([128, 128], F32, tag=f"s1t{i}")
                nc.vector.tensor_copy(st[:], pt[:])
                s1t.append(st)
            ps2 = smpsum.tile([128, H // 2], F32, tag="smp")
            for i in range(2):
                nc.tensor.matmul(ps2[:], s1t[i][:], sw2_t[i][:],
                                 start=(i == 0), stop=(i == 1))
            s2 = s1pool.tile([128, H // 2], F32, tag="s2")
            nc.vector.tensor_add(s2[:], ps2[:], sb2r_t[:])
            s2r = s1pool.tile([128, H // 2], F32, tag="s2r")
            nc.vector.tensor_scalar_max(s2r[:], s2[:], 0.0)
            qs = s1pool.tile([128, H // 2], F32, tag="qs")
            nc.vector.tensor_mul(qs[:], s2r[:], sw3r_t[:])
            nc.vector.reduce_sum(sstage[:, m:m + 1], qs[:], axis=AX.X)

        # batched output activations
        syn = consts.tile([128, MC], F32, tag="syn")
        nc.scalar.activation(syn[:], ystage[:], AF.Tanh, bias=yb2r_t[:])
        nc.sync.dma_start(out_y.ap()[:], syn[:])
        sc = consts.tile([128, MC], F32, tag="sc")
        nc.scalar.activation(sc[:], sstage[:], AF.Sigmoid, bias=sb3r_t[:])
        nc.sync.dma_start(out_s.ap()[:], sc[:])

    nc.compile()
    return nc


def _get_nc():
    if "nc" not in _CACHE:
        _CACHE["nc"] = _build_nc()
    return _CACHE["nc"]


def make_in_maps(x, gumbel, gW1, gb1, ln1_s, ln1_b, gW2, gb2, ln2_s, ln2_b,
                 gW3, gb3, sW1, sb1, sW2, sb2, sW3, sb3, yW1, yb1, yW2, yb2):
    f = np.float32

    def rep(v):
        v = np.asarray(v, f).reshape(-1)
        return np.ascontiguousarray(np.broadcast_to(v[None, :], (B, v.size)))

    xT = np.ascontiguousarray(np.asarray(x, f).T)
    g2full = np.zeros((B, MP, D), f)
    g2full[:, :M, :] = 2.0 * (np.asarray(gumbel, f)
                              + np.asarray(gb3, f).reshape(1, M, D))
    w3full = np.zeros((2, 128, MP, D), f)
    w3full[:, :, :M, :] = np.asarray(gW3, f).reshape(2, 128, M, D)
    wsy = np.ascontiguousarray(
        np.concatenate([np.asarray(sW1, f), np.asarray(yW1, f)], axis=1))

    common = dict(
        xT=xT, wsy=wsy,
        gw1=np.ascontiguousarray(np.asarray(gW1, f)),
        gw2=np.ascontiguousarray(np.asarray(gW2, f)),
        sw2=np.ascontiguousarray(np.asarray(sW2, f)),
        gb1r=rep(gb1), ln1sr=rep(ln1_s), ln1br=rep(ln1_b),
        gb2r=rep(gb2), ln2sr=rep(ln2_s), ln2br=rep(ln2_b),
        sb1yr=rep(np.concatenate([np.asarray(sb1, f), np.asarray(yb1, f)])),
        sb2r=rep(sb2), sw3r=rep(np.asarray(sW3, f)[:, 0]),
        yw2r=rep(np.asarray(yW2, f)[:, 0]),
        sb3r=np.full((B, 1), np.asarray(sb3, f).reshape(-1)[0], f),
        yb2r=np.full((B, 1), np.asarray(yb2, f).reshape(-1)[0], f),
    )
    in_maps = []
    for c in range(NCORES):
        ms = slice(c * MC, (c + 1) * MC)
        in_maps.append(dict(
            common,
            g2=np.ascontiguousarray(g2full[:, ms, :]),
            w3=np.ascontiguousarray(w3full[:, :, ms, :].transpose(2, 0, 1, 3)),
        ))
    return in_maps


def assemble(results, gb3):
    probs = np.concatenate([r["out_p"] for r in results], axis=1)[:, :M, :]
    logits = np.concatenate([r["out_l"] for r in results], axis=1)[:, :M, :]
    logits = logits + np.asarray(gb3, np.float32).reshape(1, M, D)
    scores = np.concatenate([r["out_s"] for r in results], axis=1)[:, :M]
    syn = np.concatenate([r["out_y"] for r in results], axis=1)[:, :M]
    return (np.ascontiguousarray(probs), np.ascontiguousarray(scores),
            np.ascontiguousarray(syn), np.ascontiguousarray(logits))


def kernel(**inputs):
    nc = _get_nc()
    in_maps = make_in_maps(**inputs)
    trace = bool(int(os.environ.get("KERNEL_TRACE", "0")))
    res = run_bass_kernel_spmd(nc, in_maps, core_ids=list(range(NCORES)),
                               trace=trace)
    _CACHE["last_result"] = res
    return assemble(res.results, inputs["gb3"])


if __name__ == "__main__":
    nc = _get_nc()
    print("built ok")


# revision 12
# speedup vs baseline: 3.3324x; 3.3324x over previous
"""Trainium2 Bass kernel for BiomarkerCombinationFinder.

Strategy: shard the combination axis M (padded 100->104) across 8 cores,
13 combos per core; replicate the batch (128) on SBUF partitions.  Per
combo: logits matmul on PE (fp32), gumbel-softmax stats on ACT/DVE,
top-5 via the DVE Max/MaxIndex custom ops, scorer/synergy MLPs fed by
dma_gather of the needed weight rows (2KB each) and x values.
"""

import os
import sys

sys.path.insert(0, "/opt/trn_rl_repo")

import numpy as np
from contextlib import ExitStack

import concourse.bass as bass
import concourse.bacc as bacc
import concourse.mybir as mybir
import concourse.tile as tile
from concourse.bass_utils import run_bass_kernel_spmd

B, D, H, M, K = 128, 1024, 256, 100, 5
NCORES = 8
MP = 104          # M padded to a multiple of NCORES
MC = MP // NCORES # combos per core
TAU = 0.5
EPS = 1e-5

F32 = mybir.dt.float32
I16 = mybir.dt.int16
U16 = mybir.dt.uint16

AF = mybir.ActivationFunctionType
AL = mybir.AluOpType
AX = mybir.AxisListType

_CACHE: dict = {}


def _build_nc():
    n_m = int(os.environ.get("K_NM", str(MC)))
    no_scorer = bool(int(os.environ.get("K_NOSCORER", "0")))
    no_topk = bool(int(os.environ.get("K_NOTOPK", "0")))
    no_h = bool(int(os.environ.get("K_NOH", "0")))
    no_mm = bool(int(os.environ.get("K_NOMM", "0")))
    nc = bacc.Bacc(trn_type="TRN2", num_devices=NCORES)

    # ---- DRAM I/O ----
    xT = nc.dram_tensor("xT", [D, B], F32, kind="ExternalInput")
    g2 = nc.dram_tensor("g2", [B, MC, D], F32, kind="ExternalInput")       # 2*(gumbel+gb3) slice
    w3 = nc.dram_tensor("w3", [MC, 2, 128, D], F32, kind="ExternalInput")  # gW3 slice, tiled
    wsy = nc.dram_tensor("wsy", [K * D, 2 * H], F32, kind="ExternalInput") # [sW1|yW1]
    gw1 = nc.dram_tensor("gw1", [D, 2 * H], F32, kind="ExternalInput")
    gw2 = nc.dram_tensor("gw2", [2 * H, H], F32, kind="ExternalInput")
    sw2 = nc.dram_tensor("sw2", [H, H // 2], F32, kind="ExternalInput")
    # replicated-across-partitions vectors
    gb1r = nc.dram_tensor("gb1r", [B, 2 * H], F32, kind="ExternalInput")
    ln1sr = nc.dram_tensor("ln1sr", [B, 2 * H], F32, kind="ExternalInput")
    ln1br = nc.dram_tensor("ln1br", [B, 2 * H], F32, kind="ExternalInput")
    gb2r = nc.dram_tensor("gb2r", [B, H], F32, kind="ExternalInput")
    ln2sr = nc.dram_tensor("ln2sr", [B, H], F32, kind="ExternalInput")
    ln2br = nc.dram_tensor("ln2br", [B, H], F32, kind="ExternalInput")
    sb1yr = nc.dram_tensor("sb1yr", [B, 2 * H], F32, kind="ExternalInput")
    sb2r = nc.dram_tensor("sb2r", [B, H // 2], F32, kind="ExternalInput")
    sw3r = nc.dram_tensor("sw3r", [B, H // 2], F32, kind="ExternalInput")
    yw2r = nc.dram_tensor("yw2r", [B, H], F32, kind="ExternalInput")
    sb3r = nc.dram_tensor("sb3r", [B, 1], F32, kind="ExternalInput")
    yb2r = nc.dram_tensor("yb2r", [B, 1], F32, kind="ExternalInput")

    out_l = nc.dram_tensor("out_l", [B, MC, D], F32, kind="ExternalOutput")
    out_p = nc.dram_tensor("out_p", [B, MC, D], F32, kind="ExternalOutput")
    out_s = nc.dram_tensor("out_s", [B, MC], F32, kind="ExternalOutput")
    out_y = nc.dram_tensor("out_y", [B, MC], F32, kind="ExternalOutput")

    with tile.TileContext(nc) as tc, ExitStack() as ctx:
        consts = ctx.enter_context(tc.tile_pool(name="consts", bufs=1))

        # ---- constants in SBUF ----
        ones = consts.tile([128, 640], F32, tag="ones")
        nc.vector.memset(ones[:], 1.0)

        ident = consts.tile([128, 128], F32, tag="ident")
        nc.gpsimd.affine_select(
            ident[:], ones[:, 0:128], pattern=[[1, 128]], base=0,
            channel_multiplier=-1, compare_op=AL.is_equal, fill=0.0,
        )
        # I5[p, j, d] = 1 when d == p  (for extracting vals diagonal)
        i5 = consts.tile([128, 5, 128], F32, tag="i5")
        nc.gpsimd.affine_select(
            i5[:], ones[:].rearrange("p (a b) -> p a b", a=5),
            pattern=[[0, 5], [1, 128]], base=0,
            channel_multiplier=-1, compare_op=AL.is_equal, fill=0.0,
        )
        # Prr[k, f] = 1 when k == 16*rr + f%16 (partition permutations for
        # building the 16-wrapped dma_gather index layout on-chip)
        prr = []
        for rr in range(8):
            p_t = consts.tile([128, 128], F32, tag=f"prr{rr}")
            nc.gpsimd.affine_select(
                p_t[:].rearrange("p (a b) -> p a b", b=16),
                ones[:, 0:128].rearrange("p (a b) -> p a b", b=16),
                pattern=[[0, 8], [-1, 16]], base=-16 * rr,
                channel_multiplier=1, compare_op=AL.is_equal, fill=0.0,
            )
            prr.append(p_t)
        # iota_off[p, j*8+rr] = j*1024
        iota_off = consts.tile([128, 40], I16, tag="iota_off")
        nc.gpsimd.iota(iota_off[:], pattern=[[1024, 5], [0, 8]], base=0,
                       channel_multiplier=0)

        # ---- weights/bias tiles ----
        xt_t = []
        for i in range(8):
            t_ = consts.tile([128, 128], F32, tag=f"xt{i}")
            nc.sync.dma_start(t_[:], xT.ap().rearrange("(a p) b -> a p b", p=128)[i])
            xt_t.append(t_)
        gw1_t = []
        for i in range(8):
            t_ = consts.tile([128, 2 * H], F32, tag=f"gw1_{i}")
            nc.sync.dma_start(t_[:], gw1.ap().rearrange("(a p) b -> a p b", p=128)[i])
            gw1_t.append(t_)
        gw2_t = []
        for i in range(4):
            t_ = consts.tile([128, H], F32, tag=f"gw2_{i}")
            nc.sync.dma_start(t_[:], gw2.ap().rearrange("(a p) b -> a p b", p=128)[i])
            gw2_t.append(t_)
        sw2_t = []
        for i in range(2):
            t_ = consts.tile([128, H // 2], F32, tag=f"sw2_{i}")
            nc.sync.dma_start(t_[:], sw2.ap().rearrange("(a p) b -> a p b", p=128)[i])
            sw2_t.append(t_)

        def _load(handle, shape, tag):
            t_ = consts.tile(shape, F32, tag=tag)
            nc.sync.dma_start(t_[:], handle.ap())
            return t_

        gb1r_t = _load(gb1r, [128, 2 * H], "gb1r")
        ln1sr_t = _load(ln1sr, [128, 2 * H], "ln1sr")
        ln1br_t = _load(ln1br, [128, 2 * H], "ln1br")
        gb2r_t = _load(gb2r, [128, H], "gb2r")
        ln2sr_t = _load(ln2sr, [128, H], "ln2sr")
        ln2br_t = _load(ln2br, [128, H], "ln2br")
        sb1yr_t = _load(sb1yr, [128, 2 * H], "sb1yr")
        sb2r_t = _load(sb2r, [128, H // 2], "sb2r")
        sw3r_t = _load(sw3r, [128, H // 2], "sw3r")
        yw2r_t = _load(yw2r, [128, H], "yw2r")
        sb3r_t = _load(sb3r, [128, 1], "sb3r")
        yb2r_t = _load(yb2r, [128, 1], "yb2r")

        # staging for scores / synergies (batched sigmoid/tanh at the end)
        sstage = consts.tile([128, MC], F32, tag="sstage")
        ystage = consts.tile([128, MC], F32, tag="ystage")

        epsb = consts.tile([128, 1], F32, tag="epsb")
        nc.vector.memset(epsb[:], EPS)

        # ---- generator front: h2 = relu(LN(relu(LN(x@gW1+gb1))@gW2+gb2)) ----
        def layer_norm(pool, spool, h_in, n, sr_t, br_t, tag):
            ssum = spool.tile([128, 1], F32, tag=f"{tag}_ssum")
            nc.vector.reduce_sum(ssum[:], h_in[:], axis=AX.X)
            mu = spool.tile([128, 1], F32, tag=f"{tag}_mu")
            nc.vector.tensor_scalar_mul(mu[:], ssum[:], 1.0 / n)
            hm = pool.tile([128, n], F32, tag=f"{tag}_hm")
            nc.vector.tensor_scalar(hm[:], h_in[:], mu[:], None, op0=AL.subtract)
            sq = pool.tile([128, n], F32, tag=f"{tag}_sq")
            ssq = spool.tile([128, 1], F32, tag=f"{tag}_ssq")
            nc.vector.tensor_mul(sq[:], hm[:], hm[:])
            nc.vector.reduce_sum(ssq[:], sq[:], axis=AX.X)
            std = spool.tile([128, 1], F32, tag=f"{tag}_std")
            nc.scalar.activation(std[:], ssq[:], AF.Sqrt, bias=epsb[:], scale=1.0 / n)
            rstd = spool.tile([128, 1], F32, tag=f"{tag}_rstd")
            nc.vector.reciprocal(rstd[:], std[:])
            hn = pool.tile([128, n], F32, tag=f"{tag}_hn")
            nc.vector.tensor_scalar_mul(hn[:], hm[:], rstd[:])
            hs = pool.tile([128, n], F32, tag=f"{tag}_hs")
            nc.vector.tensor_mul(hs[:], hn[:], sr_t[:])
            hb = pool.tile([128, n], F32, tag=f"{tag}_hb")
            nc.vector.tensor_add(hb[:], hs[:], br_t[:])
            hr = pool.tile([128, n], F32, tag=f"{tag}_hr")
            nc.vector.tensor_scalar_max(hr[:], hb[:], 0.0)
            return hr

        if no_h:
            h2t = []
            for i in range(2):
                st = consts.tile([128, 128], F32, tag=f"h2t{i}")
                nc.vector.memset(st[:], 0.01)
                h2t.append(st)
        else:
         with tc.tile_pool(name="hpool", bufs=1) as hpool, \
             tc.tile_pool(name="hspool", bufs=1) as hspool, \
             tc.tile_pool(name="hpsum", bufs=1, space="PSUM") as hpsum, \
             tc.tile_pool(name="htpsum", bufs=2, space="PSUM") as htpsum:
            ph1 = hpsum.tile([128, 2 * H], F32, tag="ph1")
            for i in range(8):
                nc.tensor.matmul(ph1[:], xt_t[i][:], gw1_t[i][:],
                                 start=(i == 0), stop=(i == 7))
            h1 = hpool.tile([128, 2 * H], F32, tag="h1")
            nc.vector.tensor_add(h1[:], ph1[:], gb1r_t[:])
            h1r = layer_norm(hpool, hspool, h1, 2 * H, ln1sr_t, ln1br_t, "ln1")

            h1rt = []
            for i in range(4):
                pt = htpsum.tile([128, 128], F32, tag="htp")
                nc.tensor.matmul(pt[:], h1r[:, i * 128:(i + 1) * 128], ident[:], start=True, stop=True)
                st = consts.tile([128, 128], F32, tag=f"h1rt{i}")
                nc.vector.tensor_copy(st[:], pt[:])
                h1rt.append(st)

            ph2 = hpsum.tile([128, H], F32, tag="ph2")
            for i in range(4):
                nc.tensor.matmul(ph2[:], h1rt[i][:], gw2_t[i][:],
                                 start=(i == 0), stop=(i == 3))
            h2 = hpool.tile([128, H], F32, tag="h2")
            nc.vector.tensor_add(h2[:], ph2[:], gb2r_t[:])
            h2r = layer_norm(hpool, hspool, h2, H, ln2sr_t, ln2br_t, "ln2")

            h2t = []
            for i in range(2):
                pt = htpsum.tile([128, 128], F32, tag="htp2")
                nc.tensor.matmul(pt[:], h2r[:, i * 128:(i + 1) * 128], ident[:], start=True, stop=True)
                st = consts.tile([128, 128], F32, tag=f"h2t{i}")
                nc.vector.tensor_copy(st[:], pt[:])
                h2t.append(st)

        # ---- main loop over combos ----
        wpool = ctx.enter_context(tc.tile_pool(name="wpool", bufs=3))
        gpool = ctx.enter_context(tc.tile_pool(name="gpool", bufs=3))
        lpool = ctx.enter_context(tc.tile_pool(name="lpool", bufs=2))
        tpool = ctx.enter_context(tc.tile_pool(name="tpool", bufs=2))
        epool = ctx.enter_context(tc.tile_pool(name="epool", bufs=2))
        ppool = ctx.enter_context(tc.tile_pool(name="ppool", bufs=2))
        wgpool = ctx.enter_context(tc.tile_pool(name="wgpool", bufs=3))
        xgpool = ctx.enter_context(tc.tile_pool(name="xgpool", bufs=2))
        s1pool = ctx.enter_context(tc.tile_pool(name="s1pool", bufs=2))
        spool = ctx.enter_context(tc.tile_pool(name="spool", bufs=3))
        plpsum = ctx.enter_context(tc.tile_pool(name="plpsum", bufs=2, space="PSUM"))
        smpsum = ctx.enter_context(tc.tile_pool(name="smpsum", bufs=3, space="PSUM"))

        for m in range(n_m):
            # logits = h2 @ gW3[:, m]  -> PSUM [128, 1024]
            w3t = wpool.tile([128, 2, D], F32, tag="w3t")
            for kc in range(2):
                nc.sync.dma_start(w3t[:, kc, :], w3.ap()[m, kc])
            g2t = gpool.tile([128, D], F32, tag="g2t")
            nc.sync.dma_start(g2t[:], g2.ap()[:, m, :])

            pl = plpsum.tile([128, D], F32, tag="pl")
            if no_mm:
                nc.vector.memset(pl[:], 0.25)
            else:
                for dc in range(2):
                    for kc in range(2):
                        nc.tensor.matmul(
                            pl[:, dc * 512:(dc + 1) * 512],
                            h2t[kc][:],
                            w3t[:, kc, dc * 512:(dc + 1) * 512],
                            start=(kc == 0), stop=(kc == 1),
                        )
            lsb = lpool.tile([128, D], F32, tag="lsb")
            nc.vector.tensor_copy(lsb[:], pl[:])
            nc.sync.dma_start(out_l.ap()[:, m, :], lsb[:])

            # t = 2*logits + 2*(gumbel+gb3)
            t_t = tpool.tile([128, D], F32, tag="t_t")
            nc.vector.scalar_tensor_tensor(
                t_t[:], pl[:], 2.0, g2t[:], op0=AL.mult, op1=AL.add)

            # top-8 (we use top-5)
            mx8 = spool.tile([128, 8], F32, tag="mx8")
            nc.vector.max(mx8[:], t_t[:])
            idx8 = spool.tile([128, 8], U16, tag="idx8")
            if no_topk:
                nc.vector.memset(idx8[:], 0)
            else:
                nc.vector.max_index(idx8[:], mx8[:], t_t[:])

            # softmax
            nmax = spool.tile([128, 1], F32, tag="nmax")
            nc.vector.tensor_scalar_mul(nmax[:], mx8[:, 0:1], -1.0)
            e_t = epool.tile([128, D], F32, tag="e_t")
            se = spool.tile([128, 1], F32, tag="se")
            nc.scalar.activation(e_t[:], t_t[:], AF.Exp, bias=nmax[:], accum_out=se[:])
            rc = spool.tile([128, 1], F32, tag="rc")
            nc.vector.reciprocal(rc[:], se[:])
            pb = ppool.tile([128, D], F32, tag="pb")
            nc.vector.tensor_scalar_mul(pb[:], e_t[:], rc[:])
            nc.sync.dma_start(out_p.ap()[:, m, :], pb[:])

            if no_scorer:
                nc.vector.memset(ystage[:, m:m + 1], 0.0)
                nc.vector.memset(sstage[:, m:m + 1], 0.0)
                continue

            # wrapped idx layout via one-hot permutation matmuls
            idx8f = spool.tile([128, 8], F32, tag="idx8f")
            nc.vector.tensor_copy(idx8f[:], idx8[:])
            pperm = smpsum.tile([128, 8, 8], F32, tag="smp")
            for rr in range(8):
                nc.tensor.matmul(pperm[:, rr, :], prr[rr][:], idx8f[:],
                                 start=True, stop=True)
            widx = spool.tile([128, 5, 8], I16, tag="widx")
            nc.vector.tensor_copy(
                widx[:], pperm[:].rearrange("p a b -> p b a")[:, 0:5, :])
            wpos = spool.tile([128, 40], I16, tag="wpos")
            nc.vector.tensor_add(
                wpos[:], widx[:].rearrange("p a b -> p (a b)"), iota_off[:])

            # gathers
            wg = wgpool.tile([128, 5, 2 * H], F32, tag="wg")
            nc.gpsimd.dma_gather(
                wg[:], wsy.ap(), wpos[:], num_idxs=640, num_idxs_reg=640,
                elem_size=2 * H, queue_num=0)
            xg = xgpool.tile([128, 5, 128], F32, tag="xg")
            nc.gpsimd.dma_gather(
                xg[:], xT.ap(), widx[:].rearrange("p a b -> p (a b)"),
                num_idxs=640, num_idxs_reg=640, elem_size=128, queue_num=0)

            # vals[b, j] = x[b, idx[b, j]]
            xm = xgpool.tile([128, 5, 128], F32, tag="xm")
            nc.vector.tensor_mul(xm[:], xg[:], i5[:])
            vals = spool.tile([128, 5], F32, tag="vals")
            nc.vector.reduce_sum(vals[:], xm[:], axis=AX.X)

            # s1y = relu(sum_j vals_j * W[pos_j] + [sb1|yb1])
            acc = s1pool.tile([128, 2 * H], F32, tag="acc")
            for j in range(5):
                nc.vector.scalar_tensor_tensor(
                    acc[:], wg[:, j, :], vals[:, j:j + 1],
                    (sb1yr_t[:] if j == 0 else acc[:]),
                    op0=AL.mult, op1=AL.add)
            s1y = s1pool.tile([128, 2 * H], F32, tag="s1y")
            nc.vector.tensor_scalar_max(s1y[:], acc[:], 0.0)

            # synergy head: ysum = y1 @ yW2  (per-partition dot)
            qy = s1pool.tile([128, H], F32, tag="qy")
            nc.vector.tensor_mul(qy[:], s1y[:, H:2 * H], yw2r_t[:])
            nc.vector.reduce_sum(ystage[:, m:m + 1], qy[:], axis=AX.X)

            # score head: s2 = relu(s1 @ sW2 + sb2); ssum = s2 @ sW3
            s1t = []
            for i in range(2):
                pt = smpsum.tile([128, 128], F32, tag="smp")
                nc.tensor.matmul(pt[:], s1y[:, i * 128:(i + 1) * 128], ident[:], start=True, stop=True)
                st = spool.tile([128, 128], F32, tag=f"s1t{i}")
                nc.vector.tensor_copy(st[:], pt[:])
                s1t.append(st)
            ps2 = smpsum.tile([128, H // 2], F32, tag="smp")
            for i in range(2):
                nc.tensor.matmul(ps2[:], s1t[i][:], sw2_t[i][:],
                                 start=(i == 0), stop=(i == 1))
            s2 = s1pool.tile([128, H // 2], F32, tag="s2")
            nc.vector.tensor_add(s2[:], ps2[:], sb2r_t[:])
            s2r = s1pool.tile([128, H // 2], F32, tag="s2r")
            nc.vector.tensor_scalar_max(s2r[:], s2[:], 0.0)
            qs = s1pool.tile([128, H // 2], F32, tag="qs")
            nc.vector.tensor_mul(qs[:], s2r[:], sw3r_t[:])
            nc.vector.reduce_sum(sstage[:, m:m + 1], qs[:], axis=AX.X)

        # batched output activations
        syn = consts.tile([128, MC], F32, tag="syn")
        nc.scalar.activation(syn[:], ystage[:], AF.Tanh, bias=yb2r_t[:])
        nc.sync.dma_start(out_y.ap()[:], syn[:])
        sc = consts.tile([128, MC], F32, tag="sc")
        nc.scalar.activation(sc[:], sstage[:], AF.Sigmoid, bias=sb3r_t[:])
        nc.sync.dma_start(out_s.ap()[:], sc[:])

    nc.compile()
    return nc


def _get_nc():
    if "nc" not in _CACHE:
        _CACHE["nc"] = _build_nc()
    return _CACHE["nc"]


def make_in_maps(x, gumbel, gW1, gb1, ln1_s, ln1_b, gW2, gb2, ln2_s, ln2_b,
                 gW3, gb3, sW1, sb1, sW2, sb2, sW3, sb3, yW1, yb1, yW2, yb2):
    f = np.float32

    def rep(v):
        v = np.asarray(v, f).reshape(-1)
        return np.ascontiguousarray(np.broadcast_to(v[None, :], (B, v.size)))

    xT = np.ascontiguousarray(np.asarray(x, f).T)
    g2full = np.zeros((B, MP, D), f)
    g2full[:, :M, :] = 2.0 * (np.asarray(gumbel, f)
                              + np.asarray(gb3, f).reshape(1, M, D))
    w3full = np.zeros((2, 128, MP, D), f)
    w3full[:, :, :M, :] = np.asarray(gW3, f).reshape(2, 128, M, D)
    wsy = np.ascontiguousarray(
        np.concatenate([np.asarray(sW1, f), np.asarray(yW1, f)], axis=1))

    common = dict(
        xT=xT, wsy=wsy,
        gw1=np.ascontiguousarray(np.asarray(gW1, f)),
        gw2=np.ascontiguousarray(np.asarray(gW2, f)),
        sw2=np.ascontiguousarray(np.asarray(sW2, f)),
        gb1r=rep(gb1), ln1sr=rep(ln1_s), ln1br=rep(ln1_b),
        gb2r=rep(gb2), ln2sr=rep(ln2_s), ln2br=rep(ln2_b),
        sb1yr=rep(np.concatenate([np.asarray(sb1, f), np.asarray(yb1, f)])),
        sb2r=rep(sb2), sw3r=rep(np.asarray(sW3, f)[:, 0]),
        yw2r=rep(np.asarray(yW2, f)[:, 0]),
        sb3r=np.full((B, 1), np.asarray(sb3, f).reshape(-1)[0], f),
        yb2r=np.full((B, 1), np.asarray(yb2, f).reshape(-1)[0], f),
    )
    in_maps = []
    for c in range(NCORES):
        ms = slice(c * MC, (c + 1) * MC)
        in_maps.append(dict(
            common,
            g2=np.ascontiguousarray(g2full[:, ms, :]),
            w3=np.ascontiguousarray(w3full[:, :, ms, :].transpose(2, 0, 1, 3)),
        ))
    return in_maps


def assemble(results, gb3):
    probs = np.concatenate([r["out_p"] for r in results], axis=1)[:, :M, :]
    logits = np.concatenate([r["out_l"] for r in results], axis=1)[:, :M, :]
    logits = logits + np.asarray(gb3, np.float32).reshape(1, M, D)
    scores = np.concatenate([r["out_s"] for r in results], axis=1)[:, :M]
    syn = np.concatenate([r["out_y"] for r in results], axis=1)[:, :M]
    return (np.ascontiguousarray(probs), np.ascontiguousarray(scores),
            np.ascontiguousarray(syn), np.ascontiguousarray(logits))


def kernel(**inputs):
    nc = _get_nc()
    in_maps = make_in_maps(**inputs)
    trace = bool(int(os.environ.get("KERNEL_TRACE", "0")))
    res = run_bass_kernel_spmd(nc, in_maps, core_ids=list(range(NCORES)),
                               trace=trace)
    _CACHE["last_result"] = res
    return assemble(res.results, inputs["gb3"])


if __name__ == "__main__":
    nc = _get_nc()
    print("built ok")
